# revision 3
# baseline (speedup 1.0000x reference)
"""Trainium2 Bass kernel for nn_BondDecoder (histogram_binning).

Math (derived exactly from the reference):
  a_i = 1 - src_mask ; t_i = tgt_mask ; c = a*t
  loss_b = sum_ij (a_i a_j - c_i c_j) * z_ij^2
  z = sum_h softmax_inc_h - sum_h softmax_dec_h + H_src - (g_i g_j) H_tgt

Every term carries a_i * a_j, so only unmasked (src) tokens matter. Host
compacts tokens to the first n_b positions and pads to J (=288 covers
n_b<=276 with margin; auto-rebuilds at larger J if ever exceeded). This
removes the key-mask entirely: padded k columns are exactly zero, so
padded scores are exactly 0, exp gives exactly 1, and the softmax row sum
is corrected by the host-provided constant -(J - n_b).

Device pipeline per core (4 batch elements):
  - projections as fp8(e4m3) DoubleRow matmuls (2x PE rate), folded
    conv1d+inproj weights pre-scaled by 32 so fp8 entries sit in the
    normal range; the 1/32^2 is folded into the exp scale.
  - per-head QK^T scores in fp16 into a 4-bank PSUM group; one mega-exp
    activation per 4-head group (amortizes ACT fixed overhead).
  - row sums via DVE tensor_scalar accum_out (4x mode), reciprocal once
    per [128,8] block; dec-head weights negated so z accumulates with
    adds only.
  - z assembled on PE: diag(w_g) matmuls accumulate normalized heads
    into PSUM on top of an I @ D seed; diag tiles built on GPSIMD
    (identity * w) to keep DVE free.
  - Square on ACT, then quadratic forms  u^T zsq u  and  c^T zsq c  on
    PE, final dot+reduce on DVE.
"""

from contextlib import ExitStack

import numpy as np

import concourse.bacc as bacc
import concourse.mybir as mybir
import concourse.tile as tile
from concourse.bass_utils import run_bass_kernel_spmd

L = 512
B = 32
D = 512
NCORES = 8
BPC = B // NCORES  # batch elements per core
NH = 4
HD = D // NH  # 128
JDEF = 288  # compacted+padded token count (seed-0 max n_b = 276)
S8 = 32.0  # fp8 pre-scale on folded projection weights
SCALE = float(1.0 / np.sqrt(HD) / (S8 * S8))

F8 = mybir.dt.float8e4
F16 = mybir.dt.float16
F32 = mybir.dt.float32
AF = mybir.ActivationFunctionType
ALU = mybir.AluOpType
DR = mybir.MatmulPerfMode.DoubleRow

# which proj-psum drains run on ACT (rest on DVE) — load balance knob
ACT_DRAINS = frozenset((1, 3, 5, 7, 9, 11))

_CACHE = {}


def _chunks(J):
    out = []
    i0 = 0
    while i0 < J:
        out.append((i0, min(128, J - i0)))
        i0 += 128
    return out


def _emit(ctx, tc, dram, out_ap, J, repeat=1):
    nc = tc.nc
    ics = _chunks(J)

    const_pool = ctx.enter_context(tc.tile_pool(name="const", bufs=1))
    xt_pool = ctx.enter_context(tc.tile_pool(name="xt", bufs=2))
    qk_pool = ctx.enter_context(tc.tile_pool(name="qk", bufs=2))
    e_pool = ctx.enter_context(tc.tile_pool(name="e", bufs=2))
    z_pool = ctx.enter_context(tc.tile_pool(name="z", bufs=2))
    dg_pool = ctx.enter_context(tc.tile_pool(name="dg", bufs=2))
    small_pool = ctx.enter_context(tc.tile_pool(name="small", bufs=3))
    psum_proj = ctx.enter_context(tc.tile_pool(name="pproj", bufs=2, space="PSUM"))
    psum_s = ctx.enter_context(tc.tile_pool(name="pscore", bufs=1, space="PSUM"))
    psum_z = ctx.enter_context(tc.tile_pool(name="pz", bufs=1, space="PSUM"))
    psum_q = ctx.enter_context(tc.tile_pool(name="pquad", bufs=1, space="PSUM"))

    # constants / parameters
    acat_t = []
    for e in range(2):
        t = const_pool.tile([128, 2, 4 * D], F8, tag=f"acat{e}")
        nc.sync.dma_start(t[:], dram["acat"][e])
        acat_t.append(t)
    qbr_t = const_pool.tile([128, 16], F32, tag="qbr")
    nc.sync.dma_start(qbr_t[:], dram["qbr"][:])
    i_t = const_pool.tile([128, 128], F16, tag="ident")
    nc.sync.dma_start(i_t[:], dram["ident"][:])

    for b in [b for _ in range(repeat) for b in range(BPC)]:
        # ---- per-batch loads ----
        xt_t = []
        for e in range(2):
            t = xt_pool.tile([128, 2, J], F8, tag=f"xt{e}")
            nc.sync.dma_start(t[:], dram["xt"][b, e])
            xt_t.append(t)
        d_ts = []
        for ic, (i0, pp) in enumerate(ics):
            t = z_pool.tile([128, J], F16, tag=f"d{ic}")
            nc.sync.dma_start(t[:pp], dram["dmat"][b, i0 : i0 + pp, :])
            d_ts.append(t)
        uc_t = small_pool.tile([128, 2 * len(ics)], F16, tag="uc")
        nc.sync.dma_start(uc_t[:], dram["uc"][b])
        acr_t = small_pool.tile([2, J], F32, tag="acr")
        nc.sync.dma_start(acr_t[:], dram["acr"][b])
        cn_t = small_pool.tile([128, 1], F32, tag="cn")
        nc.sync.dma_start(cn_t[:], dram["cn"][b])

        # ---- projections: fp8 DoubleRow, K=512 as 2 chained K=256 ----
        qk = []
        for dc in range(16):
            ps = psum_proj.tile([128, 512], F32, tag="pproj")
            for e in range(2):
                nc.tensor.matmul(
                    ps[:, :J],
                    acat_t[e][:, :, 128 * dc : 128 * (dc + 1)],
                    xt_t[e][:],
                    start=(e == 0),
                    stop=(e == 1),
                    perf_mode=DR,
                )
            t = qk_pool.tile([128, J], F16, tag=f"qk{dc}")
            # fold in-proj q bias on the drain (k-chain bias cancels in
            # softmax; qbr has zeros there)
            if dc in ACT_DRAINS:
                nc.scalar.activation(
                    t[:], ps[:, :J], AF.Identity, bias=qbr_t[:, dc : dc + 1]
                )
            else:
                nc.vector.tensor_scalar_add(t[:], ps[:, :J], qbr_t[:, dc : dc + 1])
            qk.append(t)

        qf = psum_q.tile([2, 512], F32, tag="pquad")
        for ic, (i0, pp) in enumerate(ics):
            # ---- scores (fp16) + mega-exp per 4-head group ----
            E = e_pool.tile([128, 8, J], F16, tag="E")
            rs = small_pool.tile([128, 8], F32, tag="rs")
            for grp in range(2):
                sc = psum_s.tile([128, 4, 512], F32, tag="pscore")
                for h in range(NH):
                    g = 4 * grp + h
                    qdc = (0 if g < 4 else 8) + (g % 4)
                    nc.tensor.matmul(
                        sc[:pp, h, :J],
                        qk[qdc][:, i0 : i0 + pp],
                        qk[qdc + 4][:],
                        start=True,
                        stop=True,
                    )
                nc.scalar.activation(
                    E[:pp, 4 * grp : 4 * grp + 4, :],
                    sc[:pp, :, :J],
                    AF.Exp,
                    scale=SCALE,
                )
            # ---- row sums (DVE, 4x mode via dummy mult) ----
            scr = e_pool.tile([128, J], F16, tag="scratch")
            for g in range(8):
                nc.vector.tensor_scalar(
                    scr[:pp],
                    E[:pp, g, :],
                    1.0,
                    0.0,
                    op0=ALU.mult,
                    op1=ALU.add,
                    accum_out=rs[:pp, g : g + 1],
                )
            # ---- pad-correct, negate dec heads, reciprocal ----
            rs2 = small_pool.tile([128, 8], F32, tag="rs2")
            nc.vector.tensor_scalar(
                rs2[:pp, 0:4], rs[:pp, 0:4], cn_t[:pp], None, op0=ALU.add
            )
            nc.vector.tensor_scalar(
                rs2[:pp, 4:8], rs[:pp, 4:8], cn_t[:pp], -1.0, op0=ALU.add, op1=ALU.mult
            )
            w = small_pool.tile([128, 8], F32, tag="w")
            nc.vector.reciprocal(w[:pp], rs2[:pp])
            # ---- diag(w_g) on GPSIMD ----
            dgs = []
            for g in range(8):
                dg = dg_pool.tile([128, 128], F16, tag=f"diag{g}")
                nc.gpsimd.tensor_scalar_mul(dg[:pp, :pp], i_t[:pp, :pp], w[:pp, g : g + 1])
                dgs.append(dg)
            # ---- z = D + sum_g diag(w_g) @ E_g  (PE, PSUM accumulate) ----
            zp = psum_z.tile([128, 512], F32, tag="pz")
            nc.tensor.matmul(
                zp[:pp, :J], i_t[:pp, :pp], d_ts[ic][:pp], start=True, stop=False
            )
            for g in range(8):
                nc.tensor.matmul(
                    zp[:pp, :J],
                    dgs[g][:pp, :pp],
                    E[:pp, g, :],
                    start=False,
                    stop=(g == 7),
                )
            zq = z_pool.tile([128, J], F16, tag="zsq")
            nc.scalar.activation(zq[:pp], zp[:pp, :J], AF.Square)
            # ---- quadratic forms: rows [u^T W ; c^T W], W = z*z ----
            nc.tensor.matmul(
                qf[:, :J],
                uc_t[:pp, 2 * ic : 2 * (ic + 1)],
                zq[:pp],
                start=(ic == 0),
                stop=(ic == len(ics) - 1),
            )
        # ---- final dots: sum_j (u^T W)_j u_j  and  -sum_j (c^T W)_j c_j ----
        fd = small_pool.tile([2, J], F32, tag="fd")
        red = small_pool.tile([2, 1], F32, tag="red")
        nc.vector.tensor_mul(fd[:], qf[:, :J], acr_t[:])
        nc.vector.tensor_reduce(red[:], fd[:], axis=mybir.AxisListType.X, op=ALU.add)
        nc.sync.dma_start(out_ap[b], red[:])


def _build(J, repeat=1):
    nc = bacc.Bacc(
        "TRN2",
        target_bir_lowering=False,
        debug=False,
        num_devices=NCORES,
    )
    nic = len(_chunks(J))
    dram = {
        "acat": nc.dram_tensor("acat", [2, 128, 2, 4 * D], F8, kind="ExternalInput").ap(),
        "qbr": nc.dram_tensor("qbr", [128, 16], F32, kind="ExternalInput").ap(),
        "ident": nc.dram_tensor("ident", [128, 128], F16, kind="ExternalInput").ap(),
        "xt": nc.dram_tensor("xt", [BPC, 2, 128, 2, J], F8, kind="ExternalInput").ap(),
        "dmat": nc.dram_tensor("dmat", [BPC, J, J], F16, kind="ExternalInput").ap(),
        "uc": nc.dram_tensor("uc", [BPC, 128, 2 * nic], F16, kind="ExternalInput").ap(),
        "acr": nc.dram_tensor("acr", [BPC, 2, J], F32, kind="ExternalInput").ap(),
        "cn": nc.dram_tensor("cn", [BPC, 128, 1], F32, kind="ExternalInput").ap(),
    }
    out_ap = nc.dram_tensor("out", [BPC, 2], F32, kind="ExternalOutput").ap()
    with tile.TileContext(nc) as tc, ExitStack() as ctx:
        _emit(ctx, tc, dram, out_ap, J, repeat=repeat)
    nc.compile()
    return nc


def get_nc(J=JDEF, repeat=1):
    key = (J, repeat)
    if key not in _CACHE:
        _CACHE[key] = _build(J, repeat=repeat)
    return _CACHE[key]


def _fold(cw, cb, W, bb):
    # q = (x @ cw.T + cb) @ W.T + bb  ==  x @ A + bias
    A = (W.astype(np.float64) @ cw.astype(np.float64)).T
    bias = cb.astype(np.float64) @ W.astype(np.float64).T + bb
    return A.astype(np.float32), bias.astype(np.float32)


def prepare_in_maps(inputs, J=None):
    np8 = mybir.dt.np(F8)
    me = np.asarray(inputs["molecule_embedding"], np.float32)  # [L, B, D]
    src_bond = np.asarray(inputs["src_bond"]).astype(np.int64)  # [B, L, 6]
    tgt_bond = np.asarray(inputs["tgt_bond"]).astype(np.int64)
    src_mask = np.asarray(inputs["src_mask"]).astype(bool)  # [B, L]
    tgt_mask = np.asarray(inputs["tgt_mask"]).astype(bool)

    idxs = [np.where(~src_mask[b])[0] for b in range(B)]
    nmax = max(len(ix) for ix in idxs)
    if J is None:
        J = JDEF if nmax <= JDEF else 32 * ((nmax + 31) // 32)
    nic = len(_chunks(J))

    A_qi, b_qi = _fold(inputs["inc_q_w"], inputs["inc_q_b"], inputs["inc_Wq"], inputs["inc_bq"])
    A_ki, _ = _fold(inputs["inc_k_w"], inputs["inc_k_b"], inputs["inc_Wk"], inputs["inc_bk"])
    A_qd, b_qd = _fold(inputs["dec_q_w"], inputs["dec_q_b"], inputs["dec_Wq"], inputs["dec_bq"])
    A_kd, _ = _fold(inputs["dec_k_w"], inputs["dec_k_b"], inputs["dec_Wk"], inputs["dec_bk"])
    acat = np.concatenate([A_qi, A_ki, A_qd, A_kd], axis=1)  # [512, 2048]
    # DoubleRow pack: logical K row kappa = 256*e + 2*p + slot
    acat8 = (acat * S8).astype(np8).reshape(2, 128, 2, 4 * D)
    # [128, 16] f32: bias for d-chunk dc in column dc (zeros for k chains)
    qbr = (
        np.concatenate([b_qi, np.zeros(D, np.float32), b_qd, np.zeros(D, np.float32)])
        .astype(np.float32)
        .reshape(16, 128)
        .T.copy()
        * S8
    )
    ident = np.eye(128, dtype=np.float16)

    t_all = tgt_mask.astype(np.float32)
    g_all = 1.0 - t_all

    # bond histograms -> D = H_src - (g_i g_j) H_tgt  (small exact integers)
    bi = np.arange(B)[:, None, None]
    li = np.arange(L)[None, :, None]
    H_s = np.zeros((B, L, L), np.float32)
    np.add.at(H_s, (bi, li, src_bond), 1.0)
    H_t = np.zeros((B, L, L), np.float32)
    np.add.at(H_t, (bi, li, tgt_bond), 1.0)
    D_full = H_s - g_all[:, :, None] * g_all[:, None, :] * H_t

    xt = np.zeros((B, 2, 128, 2, J), np8)
    dmat = np.zeros((B, J, J), np.float16)
    uc = np.zeros((B, 128, 2 * nic), np.float16)
    acr = np.zeros((B, 2, J), np.float32)
    cn = np.zeros((B, 128, 1), np.float32)
    for b in range(B):
        ix = idxs[b]
        n = len(ix)
        xp = np.zeros((D, J), np.float32)
        xp[:, :n] = me[ix, b, :].T  # [512, n]
        xt[b] = xp.astype(np8).reshape(2, 128, 2, J)
        dmat[b, :n, :n] = D_full[b][np.ix_(ix, ix)]
        u = np.zeros(J, np.float32)
        u[:n] = 1.0
        c = np.zeros(J, np.float32)
        c[:n] = t_all[b][ix]
        for ic in range(nic):
            seg = slice(128 * ic, min(128 * (ic + 1), J))
            m = seg.stop - seg.start
            uc[b, :m, 2 * ic] = u[seg]
            uc[b, :m, 2 * ic + 1] = c[seg]
        acr[b, 0] = u
        acr[b, 1] = -c
        cn[b, :, 0] = -(float(J - n))

    in_maps = []
    for cid in range(NCORES):
        sl = slice(cid * BPC, (cid + 1) * BPC)
        in_maps.append(
            {
                "acat": acat8,
                "qbr": qbr,
                "ident": ident,
                "xt": np.ascontiguousarray(xt[sl]),
                "dmat": np.ascontiguousarray(dmat[sl]),
                "uc": np.ascontiguousarray(uc[sl]),
                "acr": np.ascontiguousarray(acr[sl]),
                "cn": np.ascontiguousarray(cn[sl]),
            }
        )
    return in_maps, J


def finish(results):
    outp = np.concatenate([r["out"] for r in results], axis=0)  # [B, 2]
    return (outp[:, 0] + outp[:, 1]).astype(np.float32)


def kernel(**inputs):
    in_maps, J = prepare_in_maps(inputs)
    nc = get_nc(J)
    res = run_bass_kernel_spmd(nc, in_maps, core_ids=list(range(NCORES)))
    return finish(res.results)


if __name__ == "__main__":
    print("kernel module loaded OK")
